# revision 2
# baseline (speedup 1.0000x reference)
"""AttentiveTransformer (Dense + BN(inference) + prior-scale + sparsemax) on 8 trn2 cores.

Math (per reference):
    z   = (x @ W + b) * inv + (beta - mm*inv),  inv = gamma/sqrt(mv+eps)
    z   = z * prior_scales
    out = sparsemax(z)  (rowwise simplex projection)

Strategy (v2):
  - Host folds BN into W/bias; W and x are shipped as fp16 (PE runs fp16 at
    the same 1 cycle/row as f32r, but DMA + SBUF halve; the fp16 GEMM floor
    is rel err ~8e-4, far under the 2e-2 gate).
  - Data-parallel over batch: 8192 rows -> 8 cores x 1024 rows, 8 row-tiles
    of 128 per core.  x is host-packed per tile as [feat_part, chunk*row] so
    every DMA is 4KB/partition contiguous.
  - Matmuls: per tile 4 chunks x 16 K-steps into one 4-bank PSUM tile.
    Tiles 0/1 run chunk-major (c outer) so compute starts while W streams in
    (W-load head mostly hidden); tiles 2-7 run tile-major with W resident.
  - Drain: one DVE tensor_tensor (psum + bias -> z fp32), then per-chunk
    tensor_scalar copies z->zb (bf16) accumulating per-chunk row maxes.
  - sparsemax tau: tau0 = max(m-1, (m+s-1)/2) from top-2 of chunk maxes
    (both provable lower bounds of tau*); then r Michelot iterations on the
    bf16 copy using the identity  sum_{z>t} z = sum(max(z,t)) - (F-k)*t
    (two 4x-mode DVE passes per iter); then p exact Newton polishes with
    f = sum(relu(z - t)) on ScalarE (fp32) and k = count(z > t) on DVE
    (fp32, 2x mode).  Host verifies row sums and falls back to a high
    iteration count in the (never observed) case of non-convergence.
  - Output written as fp16 (values in [0,1]; eps 2^-11) and upcast on host.
"""

import sys

for _p in ("/opt/trn_rl_repo",):
    if _p not in sys.path:
        sys.path.append(_p)

from contextlib import ExitStack

import numpy as np

import concourse.bacc as bacc
import concourse.bass as bass
import concourse.mybir as mybir
import concourse.tile as tile
from concourse import bass_utils

B, F = 8192, 2048
NCORES = 8
BS = B // NCORES          # rows per core
P = 128                   # SBUF partitions
KC = F // P               # contraction chunks (16)
NF = 512                  # output feature chunk (one PSUM bank)
NCH = F // NF             # output chunks (4)
MT = BS // P              # row tiles per core (8)
BN_EPS = 1e-3
DEFAULT_ITERS = (2, 2)    # (bf16 rough Michelot iters, fp32 polish iters)

f32 = mybir.dt.float32
f16 = mybir.dt.float16
bf16 = mybir.dt.bfloat16


def build_program(with_prior: bool, r_rough: int, n_polish: int):
    """Build the per-core Bass program (SPMD: every core runs this)."""
    nc = bacc.Bacc()
    xt = nc.dram_tensor("xt", [BS, F], f16, kind="ExternalInput")
    wp = nc.dram_tensor("wp", [F, F], f16, kind="ExternalInput")
    bprep = nc.dram_tensor("bprep", [P, F], f32, kind="ExternalInput")
    prior = None
    if with_prior:
        prior = nc.dram_tensor("prior", [BS, F], f32, kind="ExternalInput")
    out = nc.dram_tensor("out", [BS, F], f16, kind="ExternalOutput")

    relu = mybir.ActivationFunctionType.Relu
    AO = mybir.AluOpType

    with tile.TileContext(nc) as tc, ExitStack() as ctx:
        consts = ctx.enter_context(tc.tile_pool(name="consts", bufs=1))
        wpool = ctx.enter_context(tc.tile_pool(name="w", bufs=1))
        xpool = ctx.enter_context(tc.tile_pool(name="x", bufs=4))
        zpool = ctx.enter_context(tc.tile_pool(name="z", bufs=3))
        zbpool = ctx.enter_context(tc.tile_pool(name="zb", bufs=2))
        spool = ctx.enter_context(tc.tile_pool(name="scr", bufs=2))
        opool = ctx.enter_context(tc.tile_pool(name="o", bufs=2))
        vpool = ctx.enter_context(tc.tile_pool(name="vec", bufs=8))
        psum = ctx.enter_context(tc.tile_pool(name="psum", bufs=2, space="PSUM"))
        prpool = None
        if with_prior:
            prpool = ctx.enter_context(tc.tile_pool(name="pr", bufs=2))

        # x tiles 0/1 + bias on the Activation DGE queue (W owns the SP queue)
        x_t: list = [None] * MT
        for m in (0, 1):
            xti = xpool.tile([P, F], f16, tag="xt", name=f"x{m}")
            nc.scalar.dma_start(out=xti, in_=xt[m * P:(m + 1) * P, :])
            x_t[m] = xti
        bp_t = consts.tile([P, F], f32)
        nc.scalar.dma_start(out=bp_t, in_=bprep[:, :])
        for m in (2, 3):
            xti = xpool.tile([P, F], f16, tag="xt", name=f"x{m}")
            nc.scalar.dma_start(out=xti, in_=xt[m * P:(m + 1) * P, :])
            x_t[m] = xti

        # W resident in SBUF as kc x 2 tiles of [128, 1024] fp16, streamed
        # column-pair-major so tiles 0/1 (chunk-major) can start early
        w_t = [[None] * 2 for _ in range(KC)]
        for cp in range(2):
            for k in range(KC):
                wt = wpool.tile([P, 2 * NF], f16, tag=f"w{k}_{cp}")
                nc.sync.dma_start(
                    out=wt, in_=wp[k * P:(k + 1) * P, cp * 2 * NF:(cp + 1) * 2 * NF])
                w_t[k][cp] = wt

        pr_t: list = [None] * MT

        def load_prior(m):
            if with_prior:
                prt = prpool.tile([P, F], f32, tag="pr", name=f"pr{m}")
                nc.scalar.dma_start(out=prt, in_=prior[m * P:(m + 1) * P, :])
                pr_t[m] = prt

        z_t: list = [None] * MT
        zb_t: list = [None] * MT
        mx_t: list = [None] * MT

        def alloc_tile_bufs(m):
            z_t[m] = zpool.tile([P, F], f32, tag="z", name=f"z{m}")
            zb_t[m] = zbpool.tile([P, F], bf16, tag="zb", name=f"zb{m}")
            mx_t[m] = vpool.tile([P, NCH], f32, tag="mx", name=f"mx{m}")

        def mms_chunk(m, c, ps):
            cs = slice(c * NF, (c + 1) * NF)
            hs = slice((c % 2) * NF, (c % 2 + 1) * NF)
            for k in range(KC):
                nc.tensor.matmul(
                    ps[:, cs],
                    x_t[m][:, k * P:(k + 1) * P],
                    w_t[k][c // 2][:, hs],
                    start=(k == 0),
                    stop=(k == KC - 1),
                )

        def drain(m, ps, cs):
            # z = psum + bias (fp32); prior multiply fused on the fallback path
            if with_prior:
                zt = spool.tile([P, F], f32, tag="ztmp", name=f"zt{m}")
                nc.vector.tensor_tensor(zt[:, cs], ps[:, cs], bp_t[:, cs], op=AO.add)
                nc.vector.tensor_tensor(z_t[m][:, cs], zt[:, cs], pr_t[m][:, cs],
                                        op=AO.mult)
            else:
                nc.vector.tensor_tensor(z_t[m][:, cs], ps[:, cs], bp_t[:, cs],
                                        op=AO.add)

        def maxcopy_chunk(m, c):
            # zb = bf16(z) while accumulating the chunk's row max
            cs = slice(c * NF, (c + 1) * NF)
            nc.vector.tensor_scalar(zb_t[m][:, cs], z_t[m][:, cs], 0.0, None,
                                    op0=AO.add, op1=AO.max,
                                    accum_out=mx_t[m][:, c:c + 1])

        def chain_tail(m):
            mx = mx_t[m]
            # tau0 = max(m-1, (m+s-1)/2) from top-2 of the 4 chunk maxes
            pq = vpool.tile([P, 2], f32, tag="pq", name=f"pq{m}")
            rt = vpool.tile([P, 2], f32, tag="rt", name=f"rt{m}")
            nc.vector.tensor_tensor(pq[:, 0:1], mx[:, 0:1], mx[:, 1:2], op=AO.max)
            nc.vector.tensor_tensor(pq[:, 1:2], mx[:, 0:1], mx[:, 1:2], op=AO.min)
            nc.vector.tensor_tensor(rt[:, 0:1], mx[:, 2:3], mx[:, 3:4], op=AO.max)
            nc.vector.tensor_tensor(rt[:, 1:2], mx[:, 2:3], mx[:, 3:4], op=AO.min)
            mrow = vpool.tile([P, 1], f32, tag="mrow", name=f"mr{m}")
            nc.vector.tensor_tensor(mrow, pq[:, 0:1], rt[:, 0:1], op=AO.max)
            u2 = vpool.tile([P, 1], f32, tag="u2", name=f"u2{m}")
            nc.vector.tensor_tensor(u2, pq[:, 0:1], rt[:, 0:1], op=AO.min)
            s2a = vpool.tile([P, 1], f32, tag="s2a", name=f"sa{m}")
            nc.vector.tensor_tensor(s2a, pq[:, 1:2], rt[:, 1:2], op=AO.max)
            s2 = vpool.tile([P, 1], f32, tag="s2", name=f"s2{m}")
            nc.vector.tensor_tensor(s2, s2a, u2, op=AO.max)
            b2 = vpool.tile([P, 1], f32, tag="b2", name=f"b2{m}")
            nc.vector.scalar_tensor_tensor(b2, s2, -1.0, mrow, op0=AO.add, op1=AO.add)
            nc.vector.tensor_scalar(b2, b2, 0.5, None, op0=AO.mult)
            b1 = vpool.tile([P, 1], f32, tag="b1", name=f"b1{m}")
            nc.vector.tensor_scalar(b1, mrow, -1.0, None, op0=AO.add)
            tau = vpool.tile([P, 1], f32, tag="tau", name=f"t{m}")
            nc.vector.tensor_tensor(tau, b1, b2, op=AO.max)

            # rough Michelot on bf16 copy:
            #   tau' = (smax - F*tau - 1)/k + tau,  smax = sum(max(zb,tau))
            for i in range(r_rough):
                scr_s = spool.tile([P, F], bf16, tag="ss", name=f"ss{m}_{i}")
                sacc = vpool.tile([P, 1], f32, tag="sacc", name=f"sc{m}_{i}")
                nc.vector.tensor_scalar(scr_s, zb_t[m], tau, None,
                                        op0=AO.max, op1=AO.add, accum_out=sacc)
                scr_k = spool.tile([P, F], bf16, tag="sk", name=f"sk{m}_{i}")
                kacc = vpool.tile([P, 1], f32, tag="kacc", name=f"kc{m}_{i}")
                nc.vector.tensor_scalar(scr_k, zb_t[m], tau, None,
                                        op0=AO.is_gt, op1=AO.add, accum_out=kacc)
                rk = vpool.tile([P, 1], f32, tag="rk", name=f"rk{m}_{i}")
                nc.vector.reciprocal(rk, kacc)
                t1 = vpool.tile([P, 1], f32, tag="t1", name=f"t1{m}_{i}")
                nc.vector.scalar_tensor_tensor(t1, tau, -float(F), sacc,
                                               op0=AO.mult, op1=AO.add)
                dd = vpool.tile([P, 1], f32, tag="dd", name=f"dd{m}_{i}")
                nc.vector.scalar_tensor_tensor(dd, t1, -1.0, rk,
                                               op0=AO.add, op1=AO.mult)
                tau2 = vpool.tile([P, 1], f32, tag="tau", name=f"t{m}_{i}")
                nc.vector.tensor_tensor(tau2, tau, dd, op=AO.add)
                tau = tau2

            # exact Newton polish: f from fp32 z (ScalarE), k from fp32 z (DVE)
            nt = vpool.tile([P, 1], f32, tag="nt", name=f"nt{m}")
            nc.vector.tensor_scalar(nt, tau, -1.0, None, op0=AO.mult)
            for i in range(n_polish):
                scr_f = spool.tile([P, F], f32, tag="sf", name=f"sf{m}_{i}")
                facc = vpool.tile([P, 1], f32, tag="facc", name=f"fa{m}_{i}")
                nc.scalar.activation(scr_f, z_t[m], relu, bias=nt, scale=1.0,
                                     accum_out=facc)
                scr_k2 = spool.tile([P, F], bf16, tag="sk2", name=f"k2{m}_{i}")
                kacc2 = vpool.tile([P, 1], f32, tag="kacc2", name=f"kd{m}_{i}")
                nc.vector.tensor_scalar(scr_k2, z_t[m], tau, None,
                                        op0=AO.is_gt, op1=AO.add, accum_out=kacc2)
                rk2 = vpool.tile([P, 1], f32, tag="rk2", name=f"r2{m}_{i}")
                nc.vector.reciprocal(rk2, kacc2)
                dd = vpool.tile([P, 1], f32, tag="dd2", name=f"d2{m}_{i}")
                nc.vector.scalar_tensor_tensor(dd, facc, -1.0, rk2,
                                               op0=AO.add, op1=AO.mult)
                tau2 = vpool.tile([P, 1], f32, tag="tau", name=f"tp{m}_{i}")
                nc.vector.tensor_tensor(tau2, tau, dd, op=AO.add)
                nt2 = vpool.tile([P, 1], f32, tag="nt", name=f"np{m}_{i}")
                nc.vector.tensor_tensor(nt2, nt, dd, op=AO.subtract)
                tau, nt = tau2, nt2

            # final: out = relu(z - tau) as fp16, store
            o_t = opool.tile([P, F], f16, tag="ot", name=f"o{m}")
            nc.scalar.activation(o_t, z_t[m], relu, bias=nt, scale=1.0)
            nc.sync.dma_start(out=out[m * P:(m + 1) * P, :], in_=o_t)

        # ---- phase 1: tiles 0/1 chunk-major (overlaps the W stream) ----
        ps01 = [psum.tile([P, F], f32, tag="ps", name=f"ps{m}") for m in (0, 1)]
        for m in (0, 1):
            load_prior(m)
            alloc_tile_bufs(m)
        for c in range(NCH):
            for m in (0, 1):
                mms_chunk(m, c, ps01[m])
                drain(m, ps01[m], slice(c * NF, (c + 1) * NF))
                maxcopy_chunk(m, c)
        for m in (0, 1):
            chain_tail(m)

        # ---- phase 2: tiles 2..7 tile-major (W resident) ----
        for m in range(2, MT):
            if m + 2 < MT:
                xti = xpool.tile([P, F], f16, tag="xt", name=f"x{m + 2}")
                nc.scalar.dma_start(out=xti, in_=xt[(m + 2) * P:(m + 3) * P, :])
                x_t[m + 2] = xti
            load_prior(m)
            alloc_tile_bufs(m)
            ps = psum.tile([P, F], f32, tag="ps", name=f"ps{m}")
            for c in range(NCH):
                mms_chunk(m, c, ps)
            drain(m, ps, slice(0, F))
            for c in range(NCH):
                maxcopy_chunk(m, c)
            chain_tail(m)

    nc.compile()
    return nc


_PROGRAMS: dict = {}


def _get_program(with_prior: bool, iters: tuple):
    key = (with_prior, iters)
    if key not in _PROGRAMS:
        _PROGRAMS[key] = build_program(with_prior, iters[0], iters[1])
    return _PROGRAMS[key]


def _fold_host(W, b, gamma, beta, moving_mean, moving_var):
    inv = (gamma / np.sqrt(moving_var + np.float32(BN_EPS))).astype(np.float32)
    Wp16 = np.ascontiguousarray((W * inv[None, :]).astype(np.float16))
    bp = (beta + (b - moving_mean) * inv).astype(np.float32)
    return Wp16, bp


def _prep_x(inputs):
    # xprep[core, m*128 + p, c*128 + b] = x[core*1024 + m*128 + b, c*128 + p]
    xc = inputs.reshape(NCORES, MT, P, KC, P)          # [core, m, b, c, p]
    xprep = xc.transpose(0, 1, 4, 3, 2)                # [core, m, p, c, b]
    return np.ascontiguousarray(
        xprep.astype(np.float16).reshape(NCORES, BS, F))


def _run(with_prior: bool, iters: tuple, xprep, Wp16, bp_rep, prior=None):
    nc = _get_program(with_prior, iters)
    in_maps = []
    for c in range(NCORES):
        m = {"xt": xprep[c], "wp": Wp16, "bprep": bp_rep}
        if with_prior:
            m["prior"] = np.ascontiguousarray(prior[c * BS:(c + 1) * BS, :])
        in_maps.append(m)
    res = bass_utils.run_bass_kernel_spmd(nc, in_maps, core_ids=list(range(NCORES)))
    return np.concatenate([r["out"] for r in res.results], axis=0)


def kernel(inputs, W, b, gamma, beta, moving_mean, moving_var, prior_scales):
    inputs = np.ascontiguousarray(np.asarray(inputs, dtype=np.float32))
    W = np.ascontiguousarray(np.asarray(W, dtype=np.float32))
    b = np.asarray(b, dtype=np.float32)
    gamma = np.asarray(gamma, dtype=np.float32)
    beta = np.asarray(beta, dtype=np.float32)
    moving_mean = np.asarray(moving_mean, dtype=np.float32)
    moving_var = np.asarray(moving_var, dtype=np.float32)
    prior_scales = np.asarray(prior_scales, dtype=np.float32)

    Wp16, bp = _fold_host(W, b, gamma, beta, moving_mean, moving_var)
    bp_rep = np.ascontiguousarray(np.broadcast_to(bp[None, :], (P, F)))
    xprep = _prep_x(inputs)

    # prior==1 exactly -> multiplying by it is an algebraic no-op; skip it.
    with_prior = not bool(np.all(prior_scales == np.float32(1.0)))

    out16 = _run(with_prior, DEFAULT_ITERS, xprep, Wp16, bp_rep, prior_scales)
    out = out16.astype(np.float32)

    # sparsemax rows must sum to 1; if any row is far off (never observed for
    # this data), redo with a conservative iteration count.
    rs = out.sum(axis=1, dtype=np.float64)
    if not np.all(np.abs(rs - 1.0) < 5e-3):
        out = _run(with_prior, (6, 6), xprep, Wp16, bp_rep, prior_scales)
        out = out.astype(np.float32)
    return out


# revision 4
# speedup vs baseline: 1.3116x; 1.3116x over previous
"""AttentiveTransformer (Dense + BN(inference) + prior-scale + sparsemax) on 8 trn2 cores.

Math (per reference):
    z   = (x @ W + b) * inv + (beta - mm*inv),  inv = gamma/sqrt(mv+eps)
    z   = z * prior_scales
    out = sparsemax(z)  (rowwise simplex projection)

Strategy (v3):
  - Host folds BN into W/bias; W and x ship as fp16 (PE runs fp16 at the
    same 1 cycle/row as f32r, but DMA + SBUF halve; fp16 GEMM floor is
    rel err ~8e-4 vs the 2e-2 gate).
  - Data-parallel over batch: 8192 rows -> 8 cores x 1024 rows, 8 row-tiles
    of 128 per core.  x is host-packed per tile as [feat_part, chunk*row] so
    every DMA is 4KB/partition contiguous.
  - Matmuls: per (tile, chunk) 16 K-steps into a [128,512] PSUM bank slot
    (8-deep ring).  Tiles 0-2 run chunk-major overlapping the W stream (the
    PE never waits on the 8MiB W load); tiles 3-7 run tile-major.
  - Per chunk: DVE drain (psum + bias -> z fp32) then a tensor_scalar copy
    accumulating the chunk row-max (top-2 of chunk maxes -> tau0 bounds).
  - sparsemax tau: tau0 = max(m-1, (m+s-1)/2) (provable lower bounds of
    tau*), then 3 exact Newton/Michelot steps: f = sum(relu(z-tau)) on
    ScalarE (Relu, accumulate); k = count(z>tau) on DVE (is_gt, accumulate)
    or on ScalarE via sign (k = (sum(sign(z-tau)) + F)/2), alternated to
    balance engine load.  All engines see fp32 z, so every step is exact;
    monotone Michelot convergence gives |rowsum-1| ~ 4e-3 worst-row after 3
    steps (verified on the fixed inputs host-side; host falls back to a
    high iteration count if row sums are ever off).
  - Output written as fp16 (values in [0,1]; eps 2^-11) and upcast on host.
"""

import sys

for _p in ("/opt/trn_rl_repo",):
    if _p not in sys.path:
        sys.path.append(_p)

from contextlib import ExitStack

import numpy as np

import concourse.bacc as bacc
import concourse.bass as bass
import concourse.mybir as mybir
import concourse.tile as tile
from concourse import bass_utils

B, F = 8192, 2048
NCORES = 8
BS = B // NCORES          # rows per core
P = 128                   # SBUF partitions
KC = F // P               # contraction chunks (16)
NF = 512                  # output feature chunk (one PSUM bank)
NCH = F // NF             # output chunks (4)
MT = BS // P              # row tiles per core (8)
NP1 = 3                   # tiles in the chunk-major phase (cover W stream)
BN_EPS = 1e-3
DEFAULT_ITERS = 3         # exact Newton steps (plus tau0 from top-2 bound)

f32 = mybir.dt.float32
f16 = mybir.dt.float16
bf16 = mybir.dt.bfloat16


def build_program(with_prior: bool, niters: int):
    """Build the per-core Bass program (SPMD: every core runs this)."""
    nc = bacc.Bacc()
    xt = nc.dram_tensor("xt", [BS, F], f16, kind="ExternalInput")
    wp = nc.dram_tensor("wp", [F, F], f16, kind="ExternalInput")
    bprep = nc.dram_tensor("bprep", [P, F], f32, kind="ExternalInput")
    prior = None
    if with_prior:
        prior = nc.dram_tensor("prior", [BS, F], f32, kind="ExternalInput")
    out = nc.dram_tensor("out", [BS, F], f16, kind="ExternalOutput")

    relu = mybir.ActivationFunctionType.Relu
    signf = mybir.ActivationFunctionType.Sign
    AO = mybir.AluOpType

    with tile.TileContext(nc) as tc, ExitStack() as ctx:
        consts = ctx.enter_context(tc.tile_pool(name="consts", bufs=1))
        wpool = ctx.enter_context(tc.tile_pool(name="w", bufs=1))
        xpool = ctx.enter_context(tc.tile_pool(name="x", bufs=4))
        zpool = ctx.enter_context(tc.tile_pool(name="z", bufs=3))
        spool = ctx.enter_context(tc.tile_pool(name="scr", bufs=2))
        opool = ctx.enter_context(tc.tile_pool(name="o", bufs=2))
        vpool = ctx.enter_context(tc.tile_pool(name="vec", bufs=8))
        psum = ctx.enter_context(tc.tile_pool(name="psum", bufs=8, space="PSUM"))
        prpool = None
        if with_prior:
            prpool = ctx.enter_context(tc.tile_pool(name="pr", bufs=2))

        # x tiles + bias on the Activation DGE queue (W owns the SP queue)
        x_t: list = [None] * MT

        def load_x(m):
            xti = xpool.tile([P, F], f16, tag="xt", name=f"x{m}")
            nc.scalar.dma_start(out=xti, in_=xt[m * P:(m + 1) * P, :])
            x_t[m] = xti

        for m in range(NP1):
            load_x(m)
        bp_t = consts.tile([P, F], f32)
        nc.scalar.dma_start(out=bp_t, in_=bprep[:, :])
        load_x(NP1)

        # W resident in SBUF as 16 x 2 tiles of [128, 1024] fp16, streamed
        # column-pair-major so the chunk-major phase can start immediately
        w_t = [[None] * 2 for _ in range(KC)]
        for cp in range(2):
            for k in range(KC):
                wt = wpool.tile([P, 2 * NF], f16, tag=f"w{k}_{cp}")
                nc.sync.dma_start(
                    out=wt, in_=wp[k * P:(k + 1) * P, cp * 2 * NF:(cp + 1) * 2 * NF])
                w_t[k][cp] = wt

        pr_t: list = [None] * MT

        def load_prior(m):
            if with_prior:
                prt = prpool.tile([P, F], f32, tag="pr", name=f"pr{m}")
                nc.scalar.dma_start(out=prt, in_=prior[m * P:(m + 1) * P, :])
                pr_t[m] = prt

        z_t: list = [None] * MT
        mx_t: list = [None] * MT

        def alloc_tile_bufs(m):
            z_t[m] = zpool.tile([P, F], f32, tag="z", name=f"z{m}")
            mx_t[m] = vpool.tile([P, NCH], f32, tag="mx", name=f"mx{m}")

        def chunk_mms_drain_max(m, c):
            # 16 K-step matmuls into one PSUM bank, then drain (+bias) and
            # the row-max-accumulating bf-copy of that chunk
            cs = slice(c * NF, (c + 1) * NF)
            hs = slice((c % 2) * NF, (c % 2 + 1) * NF)
            ps = psum.tile([P, NF], f32, tag="ps", name=f"ps{m}_{c}")
            for k in range(KC):
                nc.tensor.matmul(
                    ps,
                    x_t[m][:, k * P:(k + 1) * P],
                    w_t[k][c // 2][:, hs],
                    start=(k == 0),
                    stop=(k == KC - 1),
                )
            if with_prior:
                zt = spool.tile([P, NF], f32, tag="ztmp", name=f"zt{m}_{c}")
                nc.vector.tensor_tensor(zt, ps, bp_t[:, cs], op=AO.add)
                nc.vector.tensor_tensor(z_t[m][:, cs], zt, pr_t[m][:, cs],
                                        op=AO.mult)
            else:
                nc.vector.tensor_tensor(z_t[m][:, cs], ps, bp_t[:, cs], op=AO.add)
            scr_m = spool.tile([P, NF], f32, tag="sm", name=f"sm{m}_{c}")
            nc.vector.tensor_scalar(scr_m, z_t[m][:, cs], 0.0, None,
                                    op0=AO.add, op1=AO.max,
                                    accum_out=mx_t[m][:, c:c + 1])

        def chain_tail(m, count_engines):
            mx = mx_t[m]
            # tau0 = max(m-1, (m+s-1)/2) from top-2 of the 4 chunk maxes
            pq = vpool.tile([P, 2], f32, tag="pq", name=f"pq{m}")
            rt = vpool.tile([P, 2], f32, tag="rt", name=f"rt{m}")
            nc.vector.tensor_tensor(pq[:, 0:1], mx[:, 0:1], mx[:, 1:2], op=AO.max)
            nc.vector.tensor_tensor(pq[:, 1:2], mx[:, 0:1], mx[:, 1:2], op=AO.min)
            nc.vector.tensor_tensor(rt[:, 0:1], mx[:, 2:3], mx[:, 3:4], op=AO.max)
            nc.vector.tensor_tensor(rt[:, 1:2], mx[:, 2:3], mx[:, 3:4], op=AO.min)
            mrow = vpool.tile([P, 1], f32, tag="mrow", name=f"mr{m}")
            nc.vector.tensor_tensor(mrow, pq[:, 0:1], rt[:, 0:1], op=AO.max)
            u2 = vpool.tile([P, 1], f32, tag="u2", name=f"u2{m}")
            nc.vector.tensor_tensor(u2, pq[:, 0:1], rt[:, 0:1], op=AO.min)
            s2a = vpool.tile([P, 1], f32, tag="s2a", name=f"sa{m}")
            nc.vector.tensor_tensor(s2a, pq[:, 1:2], rt[:, 1:2], op=AO.max)
            s2 = vpool.tile([P, 1], f32, tag="s2", name=f"s2{m}")
            nc.vector.tensor_tensor(s2, s2a, u2, op=AO.max)
            b2 = vpool.tile([P, 2], f32, tag="b2", name=f"b2{m}")
            nc.vector.scalar_tensor_tensor(b2[:, 0:1], s2, -1.0, mrow,
                                           op0=AO.add, op1=AO.add)
            nc.vector.tensor_scalar(b2[:, 0:1], b2[:, 0:1], 0.5, None, op0=AO.mult)
            nc.vector.tensor_scalar(b2[:, 1:2], mrow, -1.0, None, op0=AO.add)
            tau = vpool.tile([P, 1], f32, tag="tau", name=f"t{m}")
            nc.vector.tensor_tensor(tau, b2[:, 1:2], b2[:, 0:1], op=AO.max)
            nt = vpool.tile([P, 1], f32, tag="nt", name=f"n{m}")
            nc.vector.tensor_scalar(nt, tau, -1.0, None, op0=AO.mult)

            # exact Newton/Michelot steps: tau' = tau + (f - 1)/k
            for i, keng in enumerate(count_engines):
                scr_f = spool.tile([P, F], f32, tag="sf", name=f"sf{m}_{i}")
                facc = vpool.tile([P, 1], f32, tag="facc", name=f"fa{m}_{i}")
                nc.scalar.activation(scr_f, z_t[m], relu, bias=nt, scale=1.0,
                                     accum_out=facc)
                kacc = vpool.tile([P, 1], f32, tag="kacc", name=f"kc{m}_{i}")
                if keng == "act":
                    scr_g = spool.tile([P, F], bf16, tag="sg", name=f"sg{m}_{i}")
                    sgn = vpool.tile([P, 1], f32, tag="sgn", name=f"sn{m}_{i}")
                    nc.scalar.activation(scr_g, z_t[m], signf, bias=nt, scale=1.0,
                                         accum_out=sgn)
                    # k = (sum(sign(z-tau)) + F)/2
                    nc.vector.tensor_scalar(kacc, sgn, float(F), 0.5,
                                            op0=AO.add, op1=AO.mult)
                else:
                    scr_k = spool.tile([P, F], bf16, tag="sk", name=f"sk{m}_{i}")
                    nc.vector.tensor_scalar(scr_k, z_t[m], tau, None,
                                            op0=AO.is_gt, op1=AO.add,
                                            accum_out=kacc)
                rk = vpool.tile([P, 1], f32, tag="rk", name=f"rk{m}_{i}")
                nc.vector.reciprocal(rk, kacc)
                dd = vpool.tile([P, 1], f32, tag="dd", name=f"dd{m}_{i}")
                nc.vector.scalar_tensor_tensor(dd, facc, -1.0, rk,
                                               op0=AO.add, op1=AO.mult)
                tau2 = vpool.tile([P, 1], f32, tag="tau", name=f"t{m}_{i}")
                nc.vector.tensor_tensor(tau2, tau, dd, op=AO.add)
                nt2 = vpool.tile([P, 1], f32, tag="nt", name=f"n{m}_{i}")
                nc.vector.tensor_tensor(nt2, nt, dd, op=AO.subtract)
                tau, nt = tau2, nt2

            # final: out = relu(z - tau) as fp16, store (SP DGE queue)
            o_t = opool.tile([P, F], f16, tag="ot", name=f"o{m}")
            nc.scalar.activation(o_t, z_t[m], relu, bias=nt, scale=1.0)
            nc.sync.dma_start(out=out[m * P:(m + 1) * P, :], in_=o_t)

        def count_engines_for(m, niters):
            # alternate the count pass between DVE and ScalarE to balance
            # engine load; the last tile keeps counts on DVE so f (ScalarE)
            # and k (DVE) overlap in the pipeline-drain tail
            if m == MT - 1:
                return ["dve"] * niters
            pat = ["dve", "act", "dve", "act", "dve", "act"]
            return pat[:niters]

        # ---- phase 1: tiles 0..NP1-1 chunk-major (hides the W stream) ----
        for m in range(NP1):
            load_prior(m)
            alloc_tile_bufs(m)
        for c in range(NCH):
            for m in range(NP1):
                chunk_mms_drain_max(m, c)
        for m in range(NP1):
            chain_tail(m, count_engines_for(m, niters))

        # ---- phase 2: tiles NP1..7 tile-major (W resident) ----
        for m in range(NP1, MT):
            for mm in (m + 1, m + 2):
                if mm < MT and x_t[mm] is None:
                    load_x(mm)
            load_prior(m)
            alloc_tile_bufs(m)
            for c in range(NCH):
                chunk_mms_drain_max(m, c)
            chain_tail(m, count_engines_for(m, niters))

    nc.compile()
    return nc


_PROGRAMS: dict = {}


def _get_program(with_prior: bool, niters: int):
    key = (with_prior, niters)
    if key not in _PROGRAMS:
        _PROGRAMS[key] = build_program(with_prior, niters)
    return _PROGRAMS[key]


def _fold_host(W, b, gamma, beta, moving_mean, moving_var):
    inv = (gamma / np.sqrt(moving_var + np.float32(BN_EPS))).astype(np.float32)
    Wp16 = np.ascontiguousarray((W * inv[None, :]).astype(np.float16))
    bp = (beta + (b - moving_mean) * inv).astype(np.float32)
    return Wp16, bp


def _prep_x(inputs):
    # xprep[core, m*128 + p, c*128 + b] = x[core*1024 + m*128 + b, c*128 + p]
    xc = inputs.reshape(NCORES, MT, P, KC, P)          # [core, m, b, c, p]
    xprep = xc.transpose(0, 1, 4, 3, 2)                # [core, m, p, c, b]
    return np.ascontiguousarray(
        xprep.astype(np.float16).reshape(NCORES, BS, F))


def _run(with_prior: bool, niters: int, xprep, Wp16, bp_rep, prior=None):
    nc = _get_program(with_prior, niters)
    in_maps = []
    for c in range(NCORES):
        m = {"xt": xprep[c], "wp": Wp16, "bprep": bp_rep}
        if with_prior:
            m["prior"] = np.ascontiguousarray(prior[c * BS:(c + 1) * BS, :])
        in_maps.append(m)
    res = bass_utils.run_bass_kernel_spmd(nc, in_maps, core_ids=list(range(NCORES)))
    return np.concatenate([r["out"] for r in res.results], axis=0)


def kernel(inputs, W, b, gamma, beta, moving_mean, moving_var, prior_scales):
    inputs = np.ascontiguousarray(np.asarray(inputs, dtype=np.float32))
    W = np.ascontiguousarray(np.asarray(W, dtype=np.float32))
    b = np.asarray(b, dtype=np.float32)
    gamma = np.asarray(gamma, dtype=np.float32)
    beta = np.asarray(beta, dtype=np.float32)
    moving_mean = np.asarray(moving_mean, dtype=np.float32)
    moving_var = np.asarray(moving_var, dtype=np.float32)
    prior_scales = np.asarray(prior_scales, dtype=np.float32)

    Wp16, bp = _fold_host(W, b, gamma, beta, moving_mean, moving_var)
    bp_rep = np.ascontiguousarray(np.broadcast_to(bp[None, :], (P, F)))
    xprep = _prep_x(inputs)

    # prior==1 exactly -> multiplying by it is an algebraic no-op; skip it.
    with_prior = not bool(np.all(prior_scales == np.float32(1.0)))

    out16 = _run(with_prior, DEFAULT_ITERS, xprep, Wp16, bp_rep, prior_scales)
    out = out16.astype(np.float32)

    # sparsemax rows must sum to 1; if any row is far off (never observed for
    # this data), redo with a conservative iteration count.
    rs = out.sum(axis=1, dtype=np.float64)
    if not np.all(np.abs(rs - 1.0) < 2e-2):
        out = _run(with_prior, 8, xprep, Wp16, bp_rep, prior_scales)
        out = out.astype(np.float32)
    return out
